# revision 1
# baseline (speedup 1.0000x reference)
"""MixHop layer (3 hops) on 8 Trainium2 NeuronCores — v2.

out = concat_j [ adj_t^j @ (x @ W_j.T + b_j) ]   for j = 0,1,2

Uses SpMM linearity:  A @ (x W + b) = (A @ x) W + (A @ 1) b, so the
per-edge gathers read raw x rows (fp16) and the dense W/b transforms run
once per 128-dest block after aggregation.

Strategy (destination sharding, one SPMD program on 8 cores):
  - Each core owns N/8 destination rows, assigned to degree-balanced
    blocks of 128 slots.  All outputs are stored block-slot-major; the
    host applies the inverse permutation after download (no device
    scatters anywhere).
  - The gather table (x, and later z2) lives in a single permuted row
    space: global row = core*SLOTS + block*128 + pos.  Hop-1 (x) and
    hop-2 (z2) therefore share identical gather index streams.
  - Segment-sum per chunk of 128 edges: one PE matmul
    psA[c,d] += G[q,c]^T S[q,d], where G is the gathered tile (lhsT) and
    S is a host-prebuilt one-hot*weight matrix streamed from DRAM
    (keeps DVE off the critical path; gathers on 4 SWDGE queues).
  - Per block after K chunks: out1T = W1.T^T gT + b1 (x) s,
    z2T = W2.T^T gT + b2 (x) s (rank-1 bias via 1-row matmuls, s = A@1
    precomputed on host from edge weights), PE-transpose z2T -> z2 rows.
  - AllGather z2 shards -> z2t table, then hop-2 segment-sum -> out2T.
Outputs y0 (dense x W0 + b0 on the own shard), o1T, o2T; host assembles
concat and un-permutes.  fp16 is used for tables/weights (PSUM stays
fp32); measured end-to-end error vs the fp32 reference is ~4e-4.
"""

import sys

sys.path.insert(0, "/opt/trn_rl_repo")

import heapq

import numpy as np

import concourse.bass as bass
import concourse.tile as tile
from concourse import bacc, mybir
from concourse import bass_utils
from concourse.masks import make_identity

P = 128


class Cfg:
    def __init__(self, n_nodes, n_feat, n_cores, k0max, k1max):
        assert n_nodes % n_cores == 0
        self.N = n_nodes
        self.F = n_feat
        self.NC = n_cores
        self.NS = n_nodes // n_cores          # dests per core
        self.NBLK = -(-self.NS // P)          # blocks per core
        self.SLOTS = self.NBLK * P            # block-slot rows per core
        self.NPAD = n_cores * self.SLOTS      # global permuted table rows
        self.K0 = k0max                       # window-0 chunks per block
        self.K1 = k1max                       # window-1 chunks per block
        self.K = k0max + k1max
        self.GM = 8                           # chunks per dma_gather
        self.NG0 = -(-(self.NBLK * k0max) // self.GM)
        self.NG1 = -(-(self.NBLK * k1max) // self.GM)
        self.WIN = 32768 if self.NPAD > 32768 else self.NPAD


def _balanced_blocks(local_dest, ns, nblk):
    """Assign dests 0..ns-1 to nblk blocks of <=P slots, balancing edge
    counts.  Returns (block_of[ns], pos_of[ns])."""
    deg = np.bincount(local_dest, minlength=ns)
    order = np.argsort(-deg, kind="stable")
    heap = [(0, 0, b) for b in range(nblk)]
    heapq.heapify(heap)
    block_of = np.empty(ns, np.int32)
    pos_of = np.empty(ns, np.int32)
    for d in order:
        while True:
            load, cnt, b = heapq.heappop(heap)
            if cnt < P:
                break
        block_of[d] = b
        pos_of[d] = cnt
        heapq.heappush(heap, (load + int(deg[d]), cnt + 1, b))
    return block_of, pos_of


def _enc_stream(idx, Kw, n_gath, nblk, GM):
    """Gather encoding [P, n_gath*GM*8]: dma_gather reads logical id i
    from [i%16, i//16] of its idx window, replicated to 8 core groups."""
    stream = idx.reshape(nblk * Kw * P)
    out = np.zeros((P, n_gath, GM * 8), np.int16)
    for g in range(n_gath):
        cg = min(GM, nblk * Kw - GM * g)
        flat = stream[g * GM * P: g * GM * P + cg * P]
        e = flat.reshape(-1, 16).T
        out[:, g, :cg * 8] = np.tile(e, (8, 1))
    return np.ascontiguousarray(out.reshape(P, n_gath * GM * 8))


def _build_program(cfg):
    N, F, NC = cfg.N, cfg.F, cfg.NC
    NBLK, K0, K1, K = cfg.NBLK, cfg.K0, cfg.K1, cfg.K
    SLOTS, NPAD, WIN = cfg.SLOTS, cfg.NPAD, cfg.WIN
    GM, NG0, NG1 = cfg.GM, cfg.NG0, cfg.NG1
    f32 = mybir.dt.float32
    f16 = mybir.dt.float16
    i16 = mybir.dt.int16

    nc = bacc.Bacc("TRN2", target_bir_lowering=False, debug=False,
                   enable_asserts=False, num_devices=NC, num_swdge_queues=4,
                   dynamic_dma_scratch_size=65536)

    # ---- inputs ----------------------------------------------------------
    x16p = nc.dram_tensor("x16p", [NPAD, F], f16, kind="ExternalInput").ap()
    xsT = nc.dram_tensor("xsT", [F, SLOTS], f16, kind="ExternalInput").ap()
    w0t = nc.dram_tensor("w0t", [F, F], f16, kind="ExternalInput").ap()
    bb0 = nc.dram_tensor("bb0", [P, F], f32, kind="ExternalInput").ap()
    w1t = nc.dram_tensor("w1t", [F, F], f16, kind="ExternalInput").ap()
    w2t = nc.dram_tensor("w2t", [F, F], f16, kind="ExternalInput").ap()
    b1r = nc.dram_tensor("b1r", [1, F], f16, kind="ExternalInput").ap()
    b2r = nc.dram_tensor("b2r", [1, F], f16, kind="ExternalInput").ap()
    s_in = nc.dram_tensor("s_in", [1, SLOTS], f16, kind="ExternalInput").ap()
    idx0_in = nc.dram_tensor("idx0", [P, NG0 * GM * 8], i16,
                             kind="ExternalInput").ap()
    idx1_in = nc.dram_tensor("idx1", [P, NG1 * GM * 8], i16,
                             kind="ExternalInput").ap()
    S_in = nc.dram_tensor("S_in", [P, NBLK * K * P], f16,
                          kind="ExternalInput").ap()

    # ---- outputs / scratch ----------------------------------------------
    y0_buf = nc.dram_tensor("y0", [SLOTS, F], f32, kind="ExternalOutput").ap()
    o1T_buf = nc.dram_tensor("o1T", [F, SLOTS], f32, kind="ExternalOutput").ap()
    o2T_buf = nc.dram_tensor("o2T", [F, SLOTS], f32, kind="ExternalOutput").ap()
    z2s = nc.dram_tensor("z2s", [SLOTS, F], f16, kind="Internal").ap()
    z2t = nc.dram_tensor("z2t", [NPAD, F], f16, kind="Internal",
                         addr_space="Shared").ap()

    with tile.TileContext(nc) as tc:
        with tc.tile_pool(name="const", bufs=1) as cpool:
            ix0_t = cpool.tile([P, NG0 * GM * 8], i16)
            nc.sync.dma_start(ix0_t[:], idx0_in[:])
            ix1_t = cpool.tile([P, NG1 * GM * 8], i16)
            nc.scalar.dma_start(ix1_t[:], idx1_in[:])
            w0_t = cpool.tile([F, F], f16, name="w0_t")
            nc.sync.dma_start(w0_t[:], w0t[:])
            bb0_t = cpool.tile([P, F], f32, name="bb0_t")
            nc.sync.dma_start(bb0_t[:], bb0[:])
            w1_t = cpool.tile([F, F], f16, name="w1_t")
            nc.sync.dma_start(w1_t[:], w1t[:])
            w2_t = cpool.tile([F, F], f16, name="w2_t")
            nc.sync.dma_start(w2_t[:], w2t[:])
            b1_t = cpool.tile([1, F], f16, name="b1_t")
            nc.sync.dma_start(b1_t[:], b1r[:])
            b2_t = cpool.tile([1, F], f16, name="b2_t")
            nc.sync.dma_start(b2_t[:], b2r[:])
            s_t = cpool.tile([1, SLOTS], f16, name="s_t")
            nc.scalar.dma_start(s_t[:], s_in[:])
            ident = cpool.tile([P, P], f16, name="ident")
            make_identity(nc, ident[:])

            # ---- Phase A: y0 = xs@W0.T + b0 (own shard) ------------------
            with tc.tile_pool(name="projA", bufs=3) as apool, \
                 tc.tile_pool(name="psumA", bufs=3, space="PSUM") as apsum:
                for t in range(NBLK):
                    r0 = t * P
                    xt = apool.tile([F, P], f16, tag="xt")
                    nc.scalar.dma_start(xt[:], xsT[:, r0:r0 + P])
                    ps0 = apsum.tile([P, F], f32, space="PSUM")
                    nc.tensor.matmul(ps0[:], lhsT=xt[:], rhs=w0_t[:],
                                     start=True, stop=True)
                    st0 = apool.tile([P, F], f32, tag="st0")
                    nc.vector.tensor_tensor(out=st0[:], in0=ps0[:],
                                            in1=bb0_t[:],
                                            op=mybir.AluOpType.add)
                    nc.scalar.dma_start(y0_buf[r0:r0 + P, :], st0[:])

            # ---- SpMM machinery ------------------------------------------
            def spmm(tab0, tab1, hop):
                with tc.tile_pool(name=f"ga{hop}", bufs=6) as gapool, \
                     tc.tile_pool(name=f"Sp{hop}", bufs=3) as spool, \
                     tc.tile_pool(name=f"post{hop}", bufs=3) as ppool, \
                     tc.tile_pool(name=f"psA{hop}", bufs=3,
                                  space="PSUM") as apsum2, \
                     tc.tile_pool(name=f"psB{hop}", bufs=2,
                                  space="PSUM") as bpsum2, \
                     tc.tile_pool(name=f"psZ{hop}", bufs=1,
                                  space="PSUM") as zpsum2:
                    wins = [[tab0, ix0_t, NBLK * K0, [], 0],
                            [tab1, ix1_t, NBLK * K1, [], 0]]
                    qctr = [0]

                    def ensure_gathers(w, upto_chunk):
                        tab, ix_t, tot, tiles, _ = wins[w]
                        while wins[w][4] * GM < min(upto_chunk, tot):
                            g = wins[w][4]
                            cg = min(GM, tot - GM * g)
                            ga = gapool.tile([P, GM, F], f16,
                                             tag=f"ga{hop}{w}",
                                             name=f"ga{hop}_{w}_{g}")
                            nc.gpsimd.dma_gather(
                                ga[:, :cg, :], tab,
                                ix_t[:, g * GM * 8: g * GM * 8 + cg * 8],
                                num_idxs=cg * P, num_idxs_reg=cg * P,
                                elem_size=F, queue_num=qctr[0] % 4)
                            qctr[0] += 1
                            tiles.append(ga)
                            wins[w][4] += 1

                    for b in range(NBLK):
                        Sb = spool.tile([P, K * P], f16, tag="Sb",
                                        name=f"Sb{hop}_{b}")
                        seng = nc.sync if b % 2 == 0 else nc.scalar
                        seng.dma_start(Sb[:], S_in[:, b * K * P:
                                                   (b + 1) * K * P])
                        ensure_gathers(0, (b + 1) * K0)
                        ensure_gathers(1, (b + 1) * K1)
                        psA = apsum2.tile([P, P], f32, space="PSUM")
                        for k in range(K):
                            if k < K0:
                                gk = b * K0 + k
                                G = wins[0][3][gk // GM][:, gk % GM, :]
                            else:
                                gk = b * K1 + (k - K0)
                                G = wins[1][3][gk // GM][:, gk % GM, :]
                            nc.tensor.matmul(psA[:], lhsT=G,
                                             rhs=Sb[:, k * P:(k + 1) * P],
                                             start=(k == 0),
                                             stop=(k == K - 1))
                        if hop == 1:
                            g1 = ppool.tile([P, P], f16, tag="g1")
                            nc.scalar.copy(g1[:], psA[:])
                            ps1 = bpsum2.tile([P, P], f32, space="PSUM")
                            nc.tensor.matmul(ps1[:], lhsT=w1_t[:], rhs=g1[:],
                                             start=True, stop=False)
                            nc.tensor.matmul(ps1[:], lhsT=b1_t[:],
                                             rhs=s_t[:, b * P:(b + 1) * P],
                                             start=False, stop=True)
                            ps2 = bpsum2.tile([P, P], f32, space="PSUM")
                            nc.tensor.matmul(ps2[:], lhsT=w2_t[:], rhs=g1[:],
                                             start=True, stop=False)
                            nc.tensor.matmul(ps2[:], lhsT=b2_t[:],
                                             rhs=s_t[:, b * P:(b + 1) * P],
                                             start=False, stop=True)
                            o1 = ppool.tile([P, P], f32, tag="o1")
                            nc.vector.tensor_copy(o1[:], ps1[:])
                            nc.scalar.dma_start(o1T_buf[:, b * P:(b + 1) * P],
                                                o1[:])
                            zT = ppool.tile([P, P], f16, tag="zT")
                            nc.scalar.copy(zT[:], ps2[:])
                            psZ = zpsum2.tile([P, P], f16, space="PSUM")
                            nc.tensor.transpose(psZ[:], zT[:], ident[:])
                            zr = ppool.tile([P, P], f16, tag="zr")
                            nc.vector.tensor_copy(zr[:], psZ[:])
                            nc.sync.dma_start(z2s[b * P:(b + 1) * P, :],
                                              zr[:])
                        else:
                            o2 = ppool.tile([P, P], f32, tag="o2")
                            nc.vector.tensor_copy(o2[:], psA[:])
                            nc.scalar.dma_start(o2T_buf[:, b * P:(b + 1) * P],
                                                o2[:])

            # ---- hop 1 over x, AllGather, hop 2 over z2 ------------------
            spmm(x16p[:WIN, :], x16p[WIN:NPAD, :], hop=1)
            nc.gpsimd.collective_compute(
                "AllGather", mybir.AluOpType.bypass,
                replica_groups=[list(range(NC))],
                ins=[z2s[:]], outs=[z2t[:]],
            )
            spmm(z2t[:WIN, :], z2t[WIN:NPAD, :], hop=2)

    nc.compile()
    return nc


_CACHE = {}


def _get_program(cfg):
    key = (cfg.N, cfg.F, cfg.NC, cfg.K0, cfg.K1)
    if key not in _CACHE:
        _CACHE[key] = _build_program(cfg)
    return _CACHE[key]


def _precompute_core(r_loc, c_slot, w, cfg, block_of, pos_of):
    """Edge encodings for one core: sort by (block, window), chunkify."""
    nblk = cfg.NBLK
    b_e = block_of[r_loc]
    d_e = pos_of[r_loc]
    win_e = (c_slot >= cfg.WIN).astype(np.int64)
    order = np.lexsort((np.arange(len(r_loc)), win_e, b_e))
    b_s, win_s, d_s, c_s, w_s = (
        b_e[order], win_e[order], d_e[order], c_slot[order], w[order])
    key = b_s * 2 + win_s
    cnt = np.bincount(key, minlength=nblk * 2).reshape(nblk, 2)
    k0 = max(1, int(np.ceil(cnt[:, 0].max() / P))) if len(r_loc) else 1
    k1 = max(1, int(np.ceil(cnt[:, 1].max() / P))) if len(r_loc) else 1
    return dict(b=b_s, win=win_s, d=d_s, c=c_s, w=w_s, cnt=cnt, k0=k0, k1=k1)


def _encode_core(pc, cfg):
    nblk, K0, K1, K = cfg.NBLK, cfg.K0, cfg.K1, cfg.K
    cnt = pc["cnt"]
    idx0 = np.zeros((nblk, K0 * P), np.int16)
    idx1 = np.zeros((nblk, K1 * P), np.int16)
    S = np.zeros((P, nblk * K * P), np.float16)
    starts = np.zeros(nblk * 2, np.int64)
    starts[1:] = np.cumsum(cnt.reshape(-1))[:-1]
    key = pc["b"] * 2 + pc["win"]
    iw = np.arange(len(key)) - starts[key]        # index within (b, win)
    b, win, d, c, w = pc["b"], pc["win"], pc["d"], pc["c"], pc["w"]
    m0 = win == 0
    idx0[b[m0], iw[m0]] = c[m0].astype(np.int16)
    m1 = ~m0
    idx1[b[m1], iw[m1]] = (c[m1] - cfg.WIN).astype(np.int16)
    kk = np.where(m0, iw // P, K0 + iw // P)      # chunk within block
    # S[q, (b*K + kk)*P + d] = w  (one-hot * weight per edge)
    S[iw % P, (b * K + kk) * P + d] = w.astype(np.float16)
    return dict(
        idx0=_enc_stream(idx0, K0, cfg.NG0, nblk, cfg.GM),
        idx1=_enc_stream(idx1, K1, cfg.NG1, nblk, cfg.GM),
        S=np.ascontiguousarray(S),
    )


def _prepare(x, edge_weight, W, b, row, col, n_cores=8):
    N, F = np.asarray(x).shape
    row = np.asarray(row).astype(np.int64)
    col = np.asarray(col).astype(np.int64)
    w = np.asarray(edge_weight).astype(np.float32)
    x = np.asarray(x).astype(np.float32)
    W = np.asarray(W).astype(np.float32)
    b = np.asarray(b).astype(np.float32)

    ns = N // n_cores
    nblk = -(-ns // P)
    slots = nblk * P
    core_of = row // ns

    # block assignment per core + global slot permutation
    slot_of = np.empty(N, np.int64)
    perms = []
    core_sel = []
    for m in range(n_cores):
        sel = np.where(core_of == m)[0]
        core_sel.append(sel)
        blk, pos = _balanced_blocks(row[sel] - m * ns, ns, nblk)
        slot_local = blk.astype(np.int64) * P + pos
        slot_of[m * ns:(m + 1) * ns] = m * slots + slot_local
        perms.append(slot_local)

    # per-core edge precompute with shared global K0/K1
    cfg0 = Cfg(N, F, n_cores, 1, 1)
    pcs = []
    for m in range(n_cores):
        sel = core_sel[m]
        r_loc = (row[sel] - m * ns).astype(np.int64)
        blk, pos = None, None  # block_of/pos_of per local dest:
        slot_local = perms[m]
        block_of = (slot_local // P).astype(np.int32)
        pos_of = (slot_local % P).astype(np.int32)
        pcs.append(_precompute_core(r_loc, slot_of[col[sel]], w[sel], cfg0,
                                    block_of, pos_of))
    k0 = max(pc["k0"] for pc in pcs)
    k1 = max(pc["k1"] for pc in pcs)
    cfg = Cfg(N, F, n_cores, k0, k1)

    # permuted fp16 gather table
    x16p = np.zeros((cfg.NPAD, F), np.float16)
    x16p[slot_of] = x.astype(np.float16)

    w0t = np.ascontiguousarray(W[0].T.astype(np.float16))
    w1t = np.ascontiguousarray(W[1].T.astype(np.float16))
    w2t = np.ascontiguousarray(W[2].T.astype(np.float16))
    bb0 = np.ascontiguousarray(np.broadcast_to(b[0][None, :], (P, F)).astype(
        np.float32))
    b1r = np.ascontiguousarray(b[1][None, :].astype(np.float16))
    b2r = np.ascontiguousarray(b[2][None, :].astype(np.float16))

    in_maps = []
    for m in range(n_cores):
        enc = _encode_core(pcs[m], cfg)
        sel = core_sel[m]
        xs = np.zeros((F, slots), np.float16)
        xs[:, :ns] = x[m * ns:(m + 1) * ns].T.astype(np.float16)
        s_host = np.bincount(slot_of[row[sel]] - m * slots, weights=w[sel],
                             minlength=slots).astype(np.float16)
        in_maps.append(dict(
            x16p=x16p, xsT=xs, w0t=w0t, bb0=bb0, w1t=w1t, w2t=w2t,
            b1r=b1r, b2r=b2r, s_in=s_host[None, :],
            idx0=enc["idx0"], idx1=enc["idx1"], S_in=enc["S"],
        ))
    return cfg, in_maps, perms


def kernel(x, edge_weight, W, b, row, col):
    n_cores = 8
    N, F = np.asarray(x).shape
    ns = N // n_cores
    cfg, in_maps, perms = _prepare(x, edge_weight, W, b, row, col, n_cores)
    nc = _get_program(cfg)
    res = bass_utils.run_bass_kernel_spmd(nc, in_maps,
                                          core_ids=list(range(n_cores)))
    outs = []
    for m in range(n_cores):
        r = res.results[m]
        slot_local = perms[m]
        y0 = r["y0"][:ns]
        o1 = np.ascontiguousarray(r["o1T"][:, slot_local].T)
        o2 = np.ascontiguousarray(r["o2T"][:, slot_local].T)
        outs.append(np.concatenate([y0, o1, o2], axis=1))
    return np.concatenate(outs, axis=0).astype(np.float32)

